# revision 8
# baseline (speedup 1.0000x reference)
"""Trainium2 Bass kernel for CausalSelfAttention (B=2, S=4096, D=512, H=8).

Sharding: 8 cores = (2 batches) x (4 head-pairs). Each core computes the
attention output for its 2 heads of its batch and the corresponding slice of
the output projection (rows of Wo), producing a partial [S, D] output. The
host sums the 4 partials per batch and adds bo (the "unshard" reduce).

Per-core pipeline (all matmuls in fp32r, 1 cycle/row on the PE):
  x[b] -> (PE transpose) x^T tiles -> Q^T, K^T, V^T projections ->
  V^T -> (PE transpose) V_aug tiles [V0|1|V1|1] ->
  for each 512-wide query block: S^T = K^T.T @ Q^T (2 heads row-packed),
  exp on ACT (scale=1/8, no max subtraction -- scores are ~N(0,1)),
  causal mask multiply on DVE for diagonal-band tiles,
  AV^T accumulation with an appended ones-column producing denominators,
  normalization via reciprocal + K=1 broadcast matmul, then the output
  projection per 128-row chunk directly from SBUF, DMA to DRAM.
"""

import sys

if "/opt/trn_rl_repo" not in sys.path:
    sys.path.insert(0, "/opt/trn_rl_repo")

import numpy as np

B, S, D, H = 2, 4096, 512, 8
HD = D // H            # 64 head dim
P = 128                # partitions
NB = 512               # query-block width
MW = 2 * HD            # 128 = columns of W per core (2 heads)

_CACHE = {}


def _build(seq_len=S):
    """Build the single-core Bass module (same program runs SPMD on 8 cores)."""
    import concourse.bass as bass  # noqa: F401
    import concourse.mybir as mybir
    import concourse.tile as tile
    from concourse import bacc
    from concourse.masks import make_identity

    f32 = mybir.dt.float32
    f32r = mybir.dt.float32r

    def r(ap):
        return ap.bitcast(f32r)

    IB = seq_len // NB     # query blocks
    SC = seq_len // P      # 128-row seq chunks (j-tiles / out chunks)
    DC = D // P            # 4 d-chunks

    nc = bacc.Bacc()
    x_ext = nc.declare_dram_parameter("x_b", [seq_len, D], f32, isOutput=False)
    wq_ext = nc.declare_dram_parameter("wq", [D, MW], f32, isOutput=False)
    wk_ext = nc.declare_dram_parameter("wk", [D, MW], f32, isOutput=False)
    wv_ext = nc.declare_dram_parameter("wv", [D, MW], f32, isOutput=False)
    wo_ext = nc.declare_dram_parameter("wo", [MW, D], f32, isOutput=False)
    bq_ext = nc.declare_dram_parameter("bq", [MW, 1], f32, isOutput=False)
    bk_ext = nc.declare_dram_parameter("bk", [MW, 1], f32, isOutput=False)
    bv_ext = nc.declare_dram_parameter("bv", [MW, 1], f32, isOutput=False)
    o_ext = nc.declare_dram_parameter("o_part", [seq_len, D], f32, isOutput=True)

    with tile.TileContext(nc) as tc:
        import contextlib

        ctx = contextlib.ExitStack()
        with ctx:
            consts = ctx.enter_context(tc.tile_pool(name="consts", bufs=1))
            persist = ctx.enter_context(tc.tile_pool(name="persist", bufs=1))
            stp = ctx.enter_context(tc.tile_pool(name="stp", bufs=2, space="PSUM"))
            avp = ctx.enter_context(tc.tile_pool(name="avp", bufs=4, space="PSUM"))

            # ---- constants -------------------------------------------------
            ident = consts.tile([P, P], f32, name="ident")
            make_identity(nc, ident[:])
            ones_f32 = consts.tile([P, HD], f32, name="ones_f32")
            nc.gpsimd.memset(ones_f32[:], 1.0)
            onesT = consts.tile([P, HD], f32r, name="onesT")
            nc.vector.tensor_copy(onesT[:], ones_f32[:])

            # 4 causal masks (multiplicative), one per 128-row offset within a
            # 512-wide query block; duplicated in both halves so one DVE mul
            # covers both heads. keep (1.0) iff j_local + 128*k <= i_local.
            masks = []
            for k in range(4):
                m = consts.tile([P, 2, NB], f32, name=f"mask{k}")
                nc.gpsimd.memset(m[:], 1.0)
                nc.gpsimd.affine_select(
                    out=m[:],
                    in_=m[:],
                    compare_op=mybir.AluOpType.is_ge,
                    fill=0.0,
                    base=-128 * k,
                    channel_multiplier=-1,
                    pattern=[[0, 2], [1, NB]],
                )
                masks.append(m)

            # ---- weights & biases -----------------------------------------
            wq_sb = consts.tile([P, DC, MW], f32r, name="wq_sb")
            wk_sb = consts.tile([P, DC, MW], f32r, name="wk_sb")
            wv_sb = consts.tile([P, DC, MW], f32r, name="wv_sb")
            for w_sb, w_ext in ((wq_sb, wq_ext), (wk_sb, wk_ext), (wv_sb, wv_ext)):
                nc.sync.dma_start(
                    out=w_sb[:],
                    in_=w_ext[:].rearrange("(dc p) m -> p dc m", p=P).bitcast(f32r),
                )
            # Wo rows split per head, both at partition base 0.
            wo_sb = [consts.tile([HD, D], f32r, name=f"wo_sb{h}") for h in range(2)]
            for h in range(2):
                nc.sync.dma_start(out=wo_sb[h][:], in_=wo_ext[h * HD:(h + 1) * HD, :].bitcast(f32r))
            bq_sb = consts.tile([MW, 1], f32, name="bq_sb")
            bk_sb = consts.tile([MW, 1], f32, name="bk_sb")
            bv_sb = consts.tile([MW, 1], f32, name="bv_sb")
            for b_sb, b_ext in ((bq_sb, bq_ext), (bk_sb, bk_ext), (bv_sb, bv_ext)):
                nc.sync.dma_start(out=b_sb[:], in_=b_ext[:])

            # ---- persistent activations -----------------------------------
            qt = [persist.tile([P, NB], f32r, name=f"qt{ib}") for ib in range(IB)]
            kt = [persist.tile([P, NB], f32r, name=f"kt{ib}") for ib in range(IB)]
            vaug = [persist.tile([P, 2 * HD + 2], f32r, name=f"vaug{jt}")
                    for jt in range(SC)]
            # attention out^T per head per block (normalized), base partition 0
            atb = [[persist.tile([HD, NB], f32r, name=f"atb{h}_{ib}")
                    for ib in range(IB)] for h in range(2)]
            for jt in range(SC):
                nc.vector.tensor_copy(vaug[jt][:, HD:HD + 1], ones_f32[:, 0:1])
                nc.vector.tensor_copy(
                    vaug[jt][:, 2 * HD + 1:2 * HD + 2], ones_f32[:, 0:1]
                )

            # ---- phase 1+2: x^T, projections, V_aug ------------------------
            with (
                tc.tile_pool(name="xtp", bufs=1) as xtp,
                tc.tile_pool(name="xnp", bufs=3) as xnp,
                tc.tile_pool(name="vtp", bufs=2) as vtp,
            ):
                # x^T tiles [d-chunk][query-block]: [128 d, 512 s]
                xt = [[xtp.tile([P, NB], f32r, name=f"xt{dc}_{ib}")
                       for ib in range(IB)] for dc in range(DC)]
                for sc in range(SC):
                    xn = xnp.tile([P, D], f32, tag="xn")
                    nc.sync.dma_start(out=xn[:], in_=x_ext[sc * P:(sc + 1) * P, :])
                    pst = avp.tile([P, NB], f32, tag="av")
                    for dc in range(DC):
                        nc.tensor.transpose(
                            pst[:, dc * P:(dc + 1) * P],
                            xn[:, dc * P:(dc + 1) * P],
                            ident[:],
                        )
                    for dc in range(DC):
                        nc.vector.tensor_copy(
                            xt[dc][sc // 4][:, (sc % 4) * P:(sc % 4 + 1) * P],
                            pst[:, dc * P:(dc + 1) * P],
                        )

                for ib in range(IB):
                    for w_sb, b_sb, dst in (
                        (wq_sb, bq_sb, qt[ib]),
                        (wk_sb, bk_sb, kt[ib]),
                    ):
                        ps = avp.tile([P, NB], f32, tag="av")
                        for dc in range(DC):
                            nc.tensor.matmul(
                                ps[:],
                                r(w_sb[:, dc, :]),
                                xt[dc][ib][:],
                                start=(dc == 0),
                                stop=(dc == DC - 1),
                            )
                        nc.vector.tensor_scalar_add(dst[:], ps[:], b_sb[:])
                    # V^T for this j-block, then transpose into V_aug tiles
                    ps = avp.tile([P, NB], f32, tag="av")
                    for dc in range(DC):
                        nc.tensor.matmul(
                            ps[:],
                            wv_sb[:, dc, :],
                            xt[dc][ib][:],
                            start=(dc == 0),
                            stop=(dc == DC - 1),
                        )
                    vt = vtp.tile([P, NB], f32, tag="vt")
                    nc.vector.tensor_scalar_add(vt[:], ps[:], bv_sb[:])
                    pstv = avp.tile([P, NB], f32, tag="av")
                    for k4 in range(4):
                        nc.tensor.transpose(
                            pstv[:, k4 * P:(k4 + 1) * P],
                            vt[:, k4 * P:(k4 + 1) * P],
                            ident[:],
                        )
                    for k4 in range(4):
                        jt = 4 * ib + k4
                        nc.vector.tensor_copy(
                            vaug[jt][:, 0:HD],
                            pstv[:, k4 * P:k4 * P + HD],
                        )
                        nc.vector.tensor_copy(
                            vaug[jt][:, HD + 1:2 * HD + 1],
                            pstv[:, k4 * P + HD:(k4 + 1) * P],
                        )

            # ---- phase 3+5: attention + output projection ------------------
            with (
                tc.tile_pool(name="ptp", bufs=3) as ptp,
                tc.tile_pool(name="recp", bufs=2) as recp,
                tc.tile_pool(name="rbp", bufs=3) as rbp,
                tc.tile_pool(name="osp", bufs=3) as osp,
            ):
                for ib in range(IB):
                    njt = 4 * (ib + 1)
                    av = [avp.tile([HD + 1, NB], f32, tag="av", name=f"av{h}_{ib}")
                          for h in range(2)]
                    for jt in range(njt):
                        st = stp.tile([P, 2 * NB], f32, tag="st")
                        for h in range(2):
                            nc.tensor.matmul(
                                st[:, h * NB:(h + 1) * NB],
                                kt[jt // 4][h * HD:(h + 1) * HD,
                                            (jt % 4) * P:(jt % 4 + 1) * P],
                                qt[ib][h * HD:(h + 1) * HD, :],
                                start=True,
                                stop=True,
                            )
                        pt = ptp.tile([P, 2 * NB], f32r, tag="pt")
                        nc.scalar.activation(
                            pt[:], st[:],
                            mybir.ActivationFunctionType.Exp,
                            scale=0.125,
                        )
                        if jt >= 4 * ib:
                            k = jt - 4 * ib
                            nc.vector.tensor_mul(
                                pt[:],
                                pt[:],
                                masks[k][:].rearrange("p a b -> p (a b)"),
                            )
                        for h in range(2):
                            nc.tensor.matmul(
                                av[h][:],
                                vaug[jt][:, h * (HD + 1):(h + 1) * (HD + 1)],
                                pt[:, h * NB:(h + 1) * NB],
                                start=(jt == 0),
                                stop=(jt == njt - 1),
                            )
                    # normalize: recip of denominators (row HD of av) and a
                    # K=1 broadcast matmul to spread them across partitions.
                    rec = recp.tile([HD + 1, 2, NB], f32r, tag="rec")
                    with nc.allow_low_precision("f32r rounding of softmax denominators"):
                        for h in range(2):
                            nc.vector.reciprocal(
                                rec[HD:HD + 1, h, :], av[h][HD:HD + 1, :]
                            )
                    for h in range(2):
                        pb = avp.tile([HD, NB], f32, tag="av", name=f"pb{h}_{ib}")
                        nc.tensor.matmul(
                            pb[:],
                            onesT[HD:HD + 1, :],
                            rec[HD:HD + 1, h, :],
                            start=True,
                            stop=True,
                        )
                        rb = rbp.tile([HD, NB], f32, tag="rb")
                        nc.vector.tensor_copy(rb[:], pb[:])
                        nc.vector.tensor_mul(
                            atb[h][ib][:], av[h][0:HD, :], rb[:]
                        )
                    # output projection for this block's 4 row-chunks
                    for k4 in range(4):
                        ic = 4 * ib + k4
                        po = avp.tile([P, NB], f32, tag="av", name=f"po{ic}")
                        for h in range(2):
                            nc.tensor.matmul(
                                po[:],
                                atb[h][ib][:, k4 * P:(k4 + 1) * P],
                                wo_sb[h][:],
                                start=(h == 0),
                                stop=(h == 1),
                            )
                        ost = osp.tile([P, NB], f32, tag="ost")
                        nc.vector.tensor_copy(ost[:], po[:])
                        nc.sync.dma_start(
                            out=o_ext[ic * P:(ic + 1) * P, :], in_=ost[:]
                        )

    nc.finalize()
    return nc


def _get_nc(seq_len=S):
    key = ("nc", seq_len)
    if key not in _CACHE:
        _CACHE[key] = _build(seq_len)
    return _CACHE[key]


def _make_in_maps(x, Wq, bq, Wk, bk, Wv, bv, Wo):
    in_maps = []
    for c in range(8):
        b, hp = c // 4, c % 4
        m0 = hp * MW
        in_maps.append({
            "x_b": np.ascontiguousarray(x[b]),
            "wq": np.ascontiguousarray(Wq[:, m0:m0 + MW]),
            "wk": np.ascontiguousarray(Wk[:, m0:m0 + MW]),
            "wv": np.ascontiguousarray(Wv[:, m0:m0 + MW]),
            "wo": np.ascontiguousarray(Wo[m0:m0 + MW, :]),
            "bq": np.ascontiguousarray(bq[m0:m0 + MW].reshape(MW, 1)),
            "bk": np.ascontiguousarray(bk[m0:m0 + MW].reshape(MW, 1)),
            "bv": np.ascontiguousarray(bv[m0:m0 + MW].reshape(MW, 1)),
        })
    return in_maps


def _run(x, Wq, bq, Wk, bk, Wv, bv, Wo, bo, **spmd_kwargs):
    from concourse.bass_utils import run_bass_kernel_spmd

    args = [np.asarray(a, dtype=np.float32)
            for a in (x, Wq, bq, Wk, bk, Wv, bv, Wo, bo)]
    x, Wq, bq, Wk, bk, Wv, bv, Wo, bo = args
    nc = _get_nc()
    in_maps = _make_in_maps(x, Wq, bq, Wk, bk, Wv, bv, Wo)
    res = run_bass_kernel_spmd(nc, in_maps, list(range(8)), **spmd_kwargs)
    out = np.empty((B, S, D), dtype=np.float32)
    for b in range(B):
        acc = res.results[4 * b]["o_part"].astype(np.float32)
        for hp in range(1, 4):
            acc = acc + res.results[4 * b + hp]["o_part"]
        out[b] = acc + bo[None, :]
    return out, res


def kernel(x, Wq, bq, Wk, bk, Wv, bv, Wo, bo):
    out, _ = _run(x, Wq, bq, Wk, bk, Wv, bv, Wo, bo)
    return out


# revision 12
# speedup vs baseline: 1.2180x; 1.2180x over previous
"""Trainium2 Bass kernel for CausalSelfAttention (B=2, S=4096, D=512, H=8).

Sharding: 8 cores = (2 batches) x (4 head-pairs). Each core computes the
attention output for its 2 heads of its batch and the corresponding slice of
the output projection (rows of Wo), producing a partial [S, D] output. The
host sums the 4 partials per batch and adds bo (the "unshard" reduce).

Per-core pipeline (all matmuls in fp32r, 1 cycle/row on the PE):
  x[b] -> (PE transpose) x^T tiles -> Q^T, K^T, V^T projections ->
  V^T -> (PE transpose) V_aug tiles [V0|1|V1|1] ->
  for each 512-wide query block: S^T = K^T.T @ Q^T (2 heads row-packed),
  exp on ACT (scale=1/8, no max subtraction -- scores are ~N(0,1)),
  causal mask multiply on DVE for diagonal-band tiles,
  AV^T accumulation with an appended ones-column producing denominators,
  normalization via reciprocal + K=1 broadcast matmul, then the output
  projection per 128-row chunk directly from SBUF, DMA to DRAM.
"""

import sys

if "/opt/trn_rl_repo" not in sys.path:
    sys.path.insert(0, "/opt/trn_rl_repo")

import numpy as np

B, S, D, H = 2, 4096, 512, 8
HD = D // H            # 64 head dim
P = 128                # partitions
NB = 512               # query-block width
MW = 2 * HD            # 128 = columns of W per core (2 heads)

_CACHE = {}


def _build(seq_len=S):
    """Build the single-core Bass module (same program runs SPMD on 8 cores)."""
    import concourse.bass as bass  # noqa: F401
    import concourse.mybir as mybir
    import concourse.tile as tile
    from concourse import bacc
    from concourse.masks import make_identity

    f32 = mybir.dt.float32
    f32r = mybir.dt.float32r

    def r(ap):
        return ap.bitcast(f32r)

    IB = seq_len // NB     # query blocks
    SC = seq_len // P      # 128-row seq chunks (j-tiles / out chunks)
    DC = D // P            # 4 d-chunks

    nc = bacc.Bacc()
    x_ext = nc.declare_dram_parameter("x_b", [seq_len, D], f32, isOutput=False)
    wq_ext = nc.declare_dram_parameter("wq", [D, MW], f32, isOutput=False)
    wk_ext = nc.declare_dram_parameter("wk", [D, MW], f32, isOutput=False)
    wv_ext = nc.declare_dram_parameter("wv", [D, MW], f32, isOutput=False)
    wo_ext = nc.declare_dram_parameter("wo", [MW, D], f32, isOutput=False)
    bq_ext = nc.declare_dram_parameter("bq", [MW, 1], f32, isOutput=False)
    bk_ext = nc.declare_dram_parameter("bk", [MW, 1], f32, isOutput=False)
    bv_ext = nc.declare_dram_parameter("bv", [MW, 1], f32, isOutput=False)
    o_ext = nc.declare_dram_parameter("o_part", [seq_len, D], f32, isOutput=True)

    with tile.TileContext(nc) as tc:
        import contextlib

        ctx = contextlib.ExitStack()
        with ctx:
            consts = ctx.enter_context(tc.tile_pool(name="consts", bufs=1))
            persist = ctx.enter_context(tc.tile_pool(name="persist", bufs=1))
            stp = ctx.enter_context(tc.tile_pool(name="stp", bufs=2, space="PSUM"))
            avp = ctx.enter_context(tc.tile_pool(name="avp", bufs=4, space="PSUM"))

            # ---- constants -------------------------------------------------
            ident = consts.tile([P, P], f32, name="ident")
            make_identity(nc, ident[:])
            ones_f32 = consts.tile([P, HD], f32, name="ones_f32")
            nc.gpsimd.memset(ones_f32[:], 1.0)
            onesT = consts.tile([P, HD], f32r, name="onesT")
            nc.vector.tensor_copy(onesT[:], ones_f32[:])

            # 4 causal masks (multiplicative), one per 128-row offset within a
            # 512-wide query block; duplicated in both halves so one DVE mul
            # covers both heads. keep (1.0) iff j_local + 128*k <= i_local.
            masks = []
            for k in range(4):
                m = consts.tile([P, 2, NB], f32, name=f"mask{k}")
                nc.gpsimd.memset(m[:], 1.0)
                nc.gpsimd.affine_select(
                    out=m[:],
                    in_=m[:],
                    compare_op=mybir.AluOpType.is_ge,
                    fill=0.0,
                    base=-128 * k,
                    channel_multiplier=-1,
                    pattern=[[0, 2], [1, NB]],
                )
                masks.append(m)

            # ---- weights & biases -----------------------------------------
            wq_sb = consts.tile([P, DC, MW], f32r, name="wq_sb")
            wk_sb = consts.tile([P, DC, MW], f32r, name="wk_sb")
            wv_sb = consts.tile([P, DC, MW], f32r, name="wv_sb")
            for w_sb, w_ext in ((wq_sb, wq_ext), (wk_sb, wk_ext), (wv_sb, wv_ext)):
                nc.sync.dma_start(
                    out=w_sb[:],
                    in_=w_ext[:].rearrange("(dc p) m -> p dc m", p=P).bitcast(f32r),
                )
            # Wo rows split per head, both at partition base 0.
            wo_sb = [consts.tile([HD, D], f32r, name=f"wo_sb{h}") for h in range(2)]
            for h in range(2):
                nc.sync.dma_start(out=wo_sb[h][:], in_=wo_ext[h * HD:(h + 1) * HD, :].bitcast(f32r))
            bq_sb = consts.tile([MW, 1], f32, name="bq_sb")
            bk_sb = consts.tile([MW, 1], f32, name="bk_sb")
            bv_sb = consts.tile([MW, 1], f32, name="bv_sb")
            for b_sb, b_ext in ((bq_sb, bq_ext), (bk_sb, bk_ext), (bv_sb, bv_ext)):
                nc.sync.dma_start(out=b_sb[:], in_=b_ext[:])

            # ---- persistent activations -----------------------------------
            qt = [persist.tile([P, NB], f32r, name=f"qt{ib}") for ib in range(IB)]
            kt = [persist.tile([P, NB], f32r, name=f"kt{ib}") for ib in range(IB)]
            vaug = [persist.tile([P, 2 * HD + 2], f32r, name=f"vaug{jt}")
                    for jt in range(SC)]
            # attention out^T per head per block (normalized), base partition 0
            atb = [[persist.tile([HD, NB], f32r, name=f"atb{h}_{ib}")
                    for ib in range(IB)] for h in range(2)]
            for jt in range(SC):
                nc.vector.tensor_copy(vaug[jt][:, HD:HD + 1], ones_f32[:, 0:1])
                nc.vector.tensor_copy(
                    vaug[jt][:, 2 * HD + 1:2 * HD + 2], ones_f32[:, 0:1]
                )

            # ---- phase 1+2: x^T, projections, V_aug ------------------------
            with (
                tc.tile_pool(name="xtp", bufs=1) as xtp,
                tc.tile_pool(name="xnp", bufs=3) as xnp,
                tc.tile_pool(name="vtp", bufs=2) as vtp,
            ):
                # x^T tiles per query-block: [128 d, dc, 512 s]
                xt = [xtp.tile([P, DC, NB], f32r, name=f"xt_{ib}")
                      for ib in range(IB)]
                for sc in range(SC):
                    xn = xnp.tile([P, D], f32, tag="xn")
                    nc.sync.dma_start(out=xn[:], in_=x_ext[sc * P:(sc + 1) * P, :])
                    pst = stp.tile([P, NB], f32, tag="st")
                    for dc in range(DC):
                        nc.tensor.transpose(
                            pst[:, dc * P:(dc + 1) * P],
                            xn[:, dc * P:(dc + 1) * P],
                            ident[:],
                        )
                    nc.vector.tensor_copy(
                        xt[sc // 4][:, :, (sc % 4) * P:(sc % 4 + 1) * P],
                        pst[:].rearrange("p (dc q) -> p dc q", dc=DC),
                    )

                for ib in range(IB):
                    for w_sb, b_sb, dst in (
                        (wq_sb, bq_sb, qt[ib]),
                        (wk_sb, bk_sb, kt[ib]),
                    ):
                        ps = avp.tile([P, NB], f32, tag="av")
                        for dc in range(DC):
                            nc.tensor.matmul(
                                ps[:],
                                w_sb[:, dc, :],
                                xt[ib][:, dc, :],
                                start=(dc == 0),
                                stop=(dc == DC - 1),
                            )
                        nc.vector.tensor_scalar_add(dst[:], ps[:], b_sb[:])
                    # V^T for this j-block, then transpose into V_aug tiles
                    ps = avp.tile([P, NB], f32, tag="av")
                    for dc in range(DC):
                        nc.tensor.matmul(
                            ps[:],
                            wv_sb[:, dc, :],
                            xt[ib][:, dc, :],
                            start=(dc == 0),
                            stop=(dc == DC - 1),
                        )
                    vt = vtp.tile([P, NB], f32, tag="vt")
                    nc.vector.tensor_scalar_add(vt[:], ps[:], bv_sb[:])
                    pstv = avp.tile([P, NB], f32, tag="av")
                    for k4 in range(4):
                        nc.tensor.transpose(
                            pstv[:, k4 * P:(k4 + 1) * P],
                            vt[:, k4 * P:(k4 + 1) * P],
                            ident[:],
                        )
                    for k4 in range(4):
                        jt = 4 * ib + k4
                        nc.vector.tensor_copy(
                            vaug[jt][:, 0:HD],
                            pstv[:, k4 * P:k4 * P + HD],
                        )
                        nc.vector.tensor_copy(
                            vaug[jt][:, HD + 1:2 * HD + 1],
                            pstv[:, k4 * P + HD:(k4 + 1) * P],
                        )

            # ---- phase 3+5: attention + output projection ------------------
            with (
                tc.tile_pool(name="ptp", bufs=3) as ptp,
                tc.tile_pool(name="recp", bufs=2) as recp,
                tc.tile_pool(name="rbp", bufs=3) as rbp,
                tc.tile_pool(name="osp", bufs=3) as osp,
            ):
                for ib in range(IB):
                    njt = 4 * (ib + 1)
                    av = [avp.tile([HD + 1, NB], f32, tag="av", name=f"av{h}_{ib}")
                          for h in range(2)]

                    def issue_st(jt):
                        st = stp.tile([P, 2 * NB], f32, tag="st")
                        for h in range(2):
                            nc.tensor.matmul(
                                st[:, h * NB:(h + 1) * NB],
                                kt[jt // 4][h * HD:(h + 1) * HD,
                                            (jt % 4) * P:(jt % 4 + 1) * P],
                                qt[ib][h * HD:(h + 1) * HD, :],
                                start=True,
                                stop=True,
                            )
                        return st

                    # software pipeline: the scores matmul for tile jt+1 is
                    # issued before the AV matmuls of tile jt, so the PE has
                    # work while ACT computes exp(jt).
                    st_next = issue_st(0)
                    for jt in range(njt):
                        st = st_next
                        if jt + 1 < njt:
                            st_next = issue_st(jt + 1)
                        pt = ptp.tile([P, 2 * NB], f32r, tag="pt")
                        nc.scalar.activation(
                            pt[:], st[:],
                            mybir.ActivationFunctionType.Exp,
                            scale=0.125,
                        )
                        if jt >= 4 * ib:
                            k = jt - 4 * ib
                            nc.vector.tensor_mul(
                                pt[:],
                                pt[:],
                                masks[k][:].rearrange("p a b -> p (a b)"),
                            )
                        for h in range(2):
                            nc.tensor.matmul(
                                av[h][:],
                                vaug[jt][:, h * (HD + 1):(h + 1) * (HD + 1)],
                                pt[:, h * NB:(h + 1) * NB],
                                start=(jt == 0),
                                stop=(jt == njt - 1),
                            )
                    # normalize: copy denominators (row HD of av) to SBUF,
                    # broadcast them across 64 partitions with a K=1 matmul,
                    # then a 64-lane-parallel approximate reciprocal.
                    rec = recp.tile([HD + 1, 2, NB], f32r, tag="rec")
                    for h in range(2):
                        nc.vector.tensor_copy(
                            rec[HD:HD + 1, h, :], av[h][HD:HD + 1, :]
                        )
                    for h in range(2):
                        pb = avp.tile([HD, NB], f32, tag="av", name=f"pb{h}_{ib}")
                        nc.tensor.matmul(
                            pb[:],
                            onesT[HD:HD + 1, :],
                            rec[HD:HD + 1, h, :],
                            start=True,
                            stop=True,
                        )
                        rb = rbp.tile([HD, NB], f32, tag="rb")
                        nc.vector.reciprocal_approx_fast(out=rb[:], in_=pb[:])
                        nc.vector.tensor_mul(
                            atb[h][ib][:], av[h][0:HD, :], rb[:]
                        )
                    # output projection for this block's 4 row-chunks
                    for k4 in range(4):
                        ic = 4 * ib + k4
                        po = avp.tile([P, NB], f32, tag="av", name=f"po{ic}")
                        for h in range(2):
                            nc.tensor.matmul(
                                po[:],
                                atb[h][ib][:, k4 * P:(k4 + 1) * P],
                                wo_sb[h][:],
                                start=(h == 0),
                                stop=(h == 1),
                            )
                        ost = osp.tile([P, NB], f32, tag="ost")
                        nc.vector.tensor_copy(ost[:], po[:])
                        nc.sync.dma_start(
                            out=o_ext[ic * P:(ic + 1) * P, :], in_=ost[:]
                        )

    nc.finalize()
    return nc


def _get_nc(seq_len=S):
    key = ("nc", seq_len)
    if key not in _CACHE:
        _CACHE[key] = _build(seq_len)
    return _CACHE[key]


def _make_in_maps(x, Wq, bq, Wk, bk, Wv, bv, Wo):
    in_maps = []
    for c in range(8):
        b, hp = c // 4, c % 4
        m0 = hp * MW
        in_maps.append({
            "x_b": np.ascontiguousarray(x[b]),
            "wq": np.ascontiguousarray(Wq[:, m0:m0 + MW]),
            "wk": np.ascontiguousarray(Wk[:, m0:m0 + MW]),
            "wv": np.ascontiguousarray(Wv[:, m0:m0 + MW]),
            "wo": np.ascontiguousarray(Wo[m0:m0 + MW, :]),
            "bq": np.ascontiguousarray(bq[m0:m0 + MW].reshape(MW, 1)),
            "bk": np.ascontiguousarray(bk[m0:m0 + MW].reshape(MW, 1)),
            "bv": np.ascontiguousarray(bv[m0:m0 + MW].reshape(MW, 1)),
        })
    return in_maps


def _run(x, Wq, bq, Wk, bk, Wv, bv, Wo, bo, **spmd_kwargs):
    from concourse.bass_utils import run_bass_kernel_spmd

    args = [np.asarray(a, dtype=np.float32)
            for a in (x, Wq, bq, Wk, bk, Wv, bv, Wo, bo)]
    x, Wq, bq, Wk, bk, Wv, bv, Wo, bo = args
    nc = _get_nc()
    in_maps = _make_in_maps(x, Wq, bq, Wk, bk, Wv, bv, Wo)
    res = run_bass_kernel_spmd(nc, in_maps, list(range(8)), **spmd_kwargs)
    out = np.empty((B, S, D), dtype=np.float32)
    for b in range(B):
        acc = res.results[4 * b]["o_part"].astype(np.float32)
        for hp in range(1, 4):
            acc = acc + res.results[4 * b + hp]["o_part"]
        out[b] = acc + bo[None, :]
    return out, res


def kernel(x, Wq, bq, Wk, bk, Wv, bv, Wo, bo):
    out, _ = _run(x, Wq, bq, Wk, bk, Wv, bv, Wo, bo)
    return out


# revision 15
# speedup vs baseline: 1.4081x; 1.1561x over previous
"""Trainium2 Bass kernel for CausalSelfAttention (B=2, S=4096, D=512, H=8).

Sharding: 8 cores = (2 batches) x (4 head-pairs). Each core computes the
attention output for its 2 heads of its batch and the corresponding slice of
the output projection (rows of Wo), producing a partial [S, D] output. The
host sums the 4 partials per batch and adds bo (the "unshard" reduce).

Per-core pipeline (all matmuls in fp32r, 1 cycle/row on the PE):
  x[b] -> (PE transpose) x^T tiles -> Q^T, K^T, V^T projections ->
  V^T -> (PE transpose) V_aug tiles [V0|1|V1|1] ->
  for each 512-wide query block: S^T = K^T.T @ Q^T (2 heads row-packed),
  exp on ACT (scale=1/8, no max subtraction -- scores are ~N(0,1)),
  causal mask multiply on DVE for diagonal-band tiles,
  AV^T accumulation with an appended ones-column producing denominators,
  normalization via reciprocal + K=1 broadcast matmul, then the output
  projection per 128-row chunk directly from SBUF, DMA to DRAM.
"""

import sys

if "/opt/trn_rl_repo" not in sys.path:
    sys.path.insert(0, "/opt/trn_rl_repo")

import numpy as np

B, S, D, H = 2, 4096, 512, 8
HD = D // H            # 64 head dim
P = 128                # partitions
NB = 512               # query-block width
MW = 2 * HD            # 128 = columns of W per core (2 heads)

_CACHE = {}


def _build(seq_len=S, prec="f32r"):
    """Build the single-core Bass module (same program runs SPMD on 8 cores)."""
    import concourse.bass as bass  # noqa: F401
    import concourse.mybir as mybir
    import concourse.tile as tile
    from concourse import bacc
    from concourse.masks import make_identity

    f32 = mybir.dt.float32
    f32r = mybir.dt.float32r
    bf16 = mybir.dt.bfloat16
    md = bf16 if prec == "bf16" else f32r

    def r(ap):
        return ap.bitcast(f32r)

    IB = seq_len // NB     # query blocks
    SC = seq_len // P      # 128-row seq chunks (j-tiles / out chunks)
    DC = D // P            # 4 d-chunks

    nc = bacc.Bacc()
    x_ext = nc.declare_dram_parameter("x_b", [seq_len, D], f32, isOutput=False)
    wq_ext = nc.declare_dram_parameter("wq", [D, MW], f32, isOutput=False)
    wk_ext = nc.declare_dram_parameter("wk", [D, MW], f32, isOutput=False)
    wv_ext = nc.declare_dram_parameter("wv", [D, MW], f32, isOutput=False)
    wo_ext = nc.declare_dram_parameter("wo", [MW, D], f32, isOutput=False)
    bq_ext = nc.declare_dram_parameter("bq", [MW, 1], f32, isOutput=False)
    bk_ext = nc.declare_dram_parameter("bk", [MW, 1], f32, isOutput=False)
    bv_ext = nc.declare_dram_parameter("bv", [MW, 1], f32, isOutput=False)
    o_ext = nc.declare_dram_parameter("o_part", [seq_len, D], f32, isOutput=True)

    with tile.TileContext(nc) as tc:
        import contextlib

        ctx = contextlib.ExitStack()
        with ctx:
            consts = ctx.enter_context(tc.tile_pool(name="consts", bufs=1))
            persist = ctx.enter_context(tc.tile_pool(name="persist", bufs=1))
            stp = ctx.enter_context(tc.tile_pool(name="stp", bufs=2, space="PSUM"))
            avp = ctx.enter_context(tc.tile_pool(name="avp", bufs=4, space="PSUM"))

            # ---- constants -------------------------------------------------
            ident = consts.tile([P, P], f32 if prec == "f32r" else md,
                                name="ident")
            make_identity(nc, ident[:])
            ones_f32 = consts.tile([P, HD], f32, name="ones_f32")
            nc.gpsimd.memset(ones_f32[:], 1.0)
            onesT = consts.tile([P, HD], md, name="onesT")
            nc.vector.tensor_copy(onesT[:], ones_f32[:])

            # 4 causal masks (multiplicative), one per 128-row offset within a
            # 512-wide query block; duplicated in both halves so one DVE mul
            # covers both heads. keep (1.0) iff j_local + 128*k <= i_local.
            masks = []
            for k in range(4):
                m = consts.tile([P, 2, NB], md, name=f"mask{k}")
                nc.gpsimd.memset(m[:], 1.0)
                nc.gpsimd.affine_select(
                    out=m[:],
                    in_=m[:],
                    compare_op=mybir.AluOpType.is_ge,
                    fill=0.0,
                    base=-128 * k,
                    channel_multiplier=-1,
                    pattern=[[0, 2], [1, NB]],
                )
                masks.append(m)

            # ---- weights & biases -----------------------------------------
            wq_sb = consts.tile([P, DC, MW], md, name="wq_sb")
            wk_sb = consts.tile([P, DC, MW], md, name="wk_sb")
            wv_sb = consts.tile([P, DC, MW], md, name="wv_sb")
            wo_sb = [consts.tile([HD, D], md, name=f"wo_sb{h}") for h in range(2)]
            if prec == "f32r":
                for w_sb, w_ext in ((wq_sb, wq_ext), (wk_sb, wk_ext), (wv_sb, wv_ext)):
                    nc.sync.dma_start(
                        out=w_sb[:],
                        in_=w_ext[:].rearrange("(dc p) m -> p dc m", p=P).bitcast(f32r),
                    )
                for h in range(2):
                    nc.sync.dma_start(
                        out=wo_sb[h][:],
                        in_=wo_ext[h * HD:(h + 1) * HD, :].bitcast(f32r),
                    )
            else:
                for w_sb, w_ext in ((wq_sb, wq_ext), (wk_sb, wk_ext), (wv_sb, wv_ext)):
                    ws = consts.tile([P, DC, MW], f32, name=f"ws_{w_ext.name}")
                    nc.sync.dma_start(
                        out=ws[:],
                        in_=w_ext[:].rearrange("(dc p) m -> p dc m", p=P),
                    )
                    nc.vector.tensor_copy(w_sb[:], ws[:])
                wos = consts.tile([MW, D], f32, name="wos")
                nc.sync.dma_start(out=wos[:], in_=wo_ext[:])
                for h in range(2):
                    nc.vector.tensor_copy(
                        wo_sb[h][:], wos[h * HD:(h + 1) * HD, :]
                    )
            bq_sb = consts.tile([MW, 1], f32, name="bq_sb")
            bk_sb = consts.tile([MW, 1], f32, name="bk_sb")
            bv_sb = consts.tile([MW, 1], f32, name="bv_sb")
            for b_sb, b_ext in ((bq_sb, bq_ext), (bk_sb, bk_ext), (bv_sb, bv_ext)):
                nc.sync.dma_start(out=b_sb[:], in_=b_ext[:])

            # ---- persistent activations -----------------------------------
            qt = [persist.tile([P, NB], md, name=f"qt{ib}") for ib in range(IB)]
            kt = [persist.tile([P, NB], md, name=f"kt{ib}") for ib in range(IB)]
            vaug = [persist.tile([P, 2 * HD + 2], md, name=f"vaug{jt}")
                    for jt in range(SC)]
            # attention out^T per head per block (normalized), base partition 0
            atb = [[persist.tile([HD, NB], md, name=f"atb{h}_{ib}")
                    for ib in range(IB)] for h in range(2)]
            for jt in range(SC):
                nc.vector.tensor_copy(vaug[jt][:, HD:HD + 1], ones_f32[:, 0:1])
                nc.vector.tensor_copy(
                    vaug[jt][:, 2 * HD + 1:2 * HD + 2], ones_f32[:, 0:1]
                )

            # ---- phase 1+2: x^T, projections, V_aug ------------------------
            with (
                tc.tile_pool(name="xtp", bufs=1) as xtp,
                tc.tile_pool(name="xnp", bufs=3) as xnp,
                tc.tile_pool(name="vtp", bufs=2) as vtp,
            ):
                # x^T tiles per query-block: [128 d, dc, 512 s]
                xt = [xtp.tile([P, DC, NB], md, name=f"xt_{ib}")
                      for ib in range(IB)]
                for sc in range(SC):
                    xn = xnp.tile([P, D], f32, tag="xn")
                    nc.sync.dma_start(out=xn[:], in_=x_ext[sc * P:(sc + 1) * P, :])
                    if prec == "bf16":
                        xnc = xnp.tile([P, D], md, tag="xnc")
                        nc.vector.tensor_copy(xnc[:], xn[:])
                    else:
                        xnc = xn
                    pst = stp.tile([P, NB], f32 if prec == "f32r" else md,
                                   tag="st")
                    for dc in range(DC):
                        nc.tensor.transpose(
                            pst[:, dc * P:(dc + 1) * P],
                            xnc[:, dc * P:(dc + 1) * P],
                            ident[:],
                        )
                    nc.vector.tensor_copy(
                        xt[sc // 4][:, :, (sc % 4) * P:(sc % 4 + 1) * P],
                        pst[:].rearrange("p (dc q) -> p dc q", dc=DC),
                    )

                for ib in range(IB):
                    for w_sb, b_sb, dst in (
                        (wq_sb, bq_sb, qt[ib]),
                        (wk_sb, bk_sb, kt[ib]),
                    ):
                        ps = avp.tile([P, NB], f32, tag="av")
                        for dc in range(DC):
                            nc.tensor.matmul(
                                ps[:],
                                w_sb[:, dc, :],
                                xt[ib][:, dc, :],
                                start=(dc == 0),
                                stop=(dc == DC - 1),
                            )
                        nc.vector.tensor_scalar_add(dst[:], ps[:], b_sb[:])
                    # V^T for this j-block, then transpose into V_aug tiles
                    ps = avp.tile([P, NB], f32, tag="av")
                    for dc in range(DC):
                        nc.tensor.matmul(
                            ps[:],
                            wv_sb[:, dc, :],
                            xt[ib][:, dc, :],
                            start=(dc == 0),
                            stop=(dc == DC - 1),
                        )
                    vt = vtp.tile([P, NB], f32 if prec == "f32r" else md,
                                  tag="vt")
                    nc.vector.tensor_scalar_add(vt[:], ps[:], bv_sb[:])
                    pstv = avp.tile([P, NB], f32 if prec == "f32r" else md,
                                    tag="av")
                    for k4 in range(4):
                        nc.tensor.transpose(
                            pstv[:, k4 * P:(k4 + 1) * P],
                            vt[:, k4 * P:(k4 + 1) * P],
                            ident[:],
                        )
                    for k4 in range(4):
                        jt = 4 * ib + k4
                        nc.vector.tensor_copy(
                            vaug[jt][:, 0:HD],
                            pstv[:, k4 * P:k4 * P + HD],
                        )
                        nc.vector.tensor_copy(
                            vaug[jt][:, HD + 1:2 * HD + 1],
                            pstv[:, k4 * P + HD:(k4 + 1) * P],
                        )

            # ---- phase 3+5: attention + output projection ------------------
            with (
                tc.tile_pool(name="ptp", bufs=3) as ptp,
                tc.tile_pool(name="recp", bufs=2) as recp,
                tc.tile_pool(name="rbp", bufs=3) as rbp,
                tc.tile_pool(name="osp", bufs=3) as osp,
            ):
                for ib in range(IB):
                    njt = 4 * (ib + 1)
                    av = [avp.tile([HD + 1, NB], f32, tag="av", name=f"av{h}_{ib}")
                          for h in range(2)]

                    def issue_st(jt):
                        st = stp.tile([P, 2 * NB], f32, tag="st")
                        for h in range(2):
                            nc.tensor.matmul(
                                st[:, h * NB:(h + 1) * NB],
                                kt[jt // 4][h * HD:(h + 1) * HD,
                                            (jt % 4) * P:(jt % 4 + 1) * P],
                                qt[ib][h * HD:(h + 1) * HD, :],
                                start=True,
                                stop=True,
                            )
                        return st

                    # software pipeline: the scores matmul for tile jt+1 is
                    # issued before the AV matmuls of tile jt, so the PE has
                    # work while ACT computes exp(jt).
                    st_next = issue_st(0)
                    for jt in range(njt):
                        st = st_next
                        if jt + 1 < njt:
                            st_next = issue_st(jt + 1)
                        pt = ptp.tile([P, 2 * NB], md, tag="pt")
                        nc.scalar.activation(
                            pt[:], st[:],
                            mybir.ActivationFunctionType.Exp,
                            scale=0.125,
                        )
                        if jt >= 4 * ib:
                            k = jt - 4 * ib
                            nc.vector.tensor_mul(
                                pt[:],
                                pt[:],
                                masks[k][:].rearrange("p a b -> p (a b)"),
                            )
                        for h in range(2):
                            nc.tensor.matmul(
                                av[h][:],
                                vaug[jt][:, h * (HD + 1):(h + 1) * (HD + 1)],
                                pt[:, h * NB:(h + 1) * NB],
                                start=(jt == 0),
                                stop=(jt == njt - 1),
                            )
                    # normalize: copy denominators (row HD of av) to SBUF,
                    # broadcast them across 64 partitions with a K=1 matmul,
                    # then a 64-lane-parallel approximate reciprocal.
                    rec = recp.tile([HD + 1, 2, NB], md, tag="rec")
                    for h in range(2):
                        nc.vector.tensor_copy(
                            rec[HD:HD + 1, h, :], av[h][HD:HD + 1, :]
                        )
                    for h in range(2):
                        pb = avp.tile([HD, NB], f32, tag="av", name=f"pb{h}_{ib}")
                        nc.tensor.matmul(
                            pb[:],
                            onesT[HD:HD + 1, :],
                            rec[HD:HD + 1, h, :],
                            start=True,
                            stop=True,
                        )
                        rb = rbp.tile([HD, NB], f32, tag="rb")
                        nc.vector.reciprocal_approx_fast(out=rb[:], in_=pb[:])
                        nc.vector.tensor_mul(
                            atb[h][ib][:], av[h][0:HD, :], rb[:]
                        )
                    # output projection for this block's 4 row-chunks
                    for k4 in range(4):
                        ic = 4 * ib + k4
                        po = avp.tile([P, NB], f32, tag="av", name=f"po{ic}")
                        for h in range(2):
                            nc.tensor.matmul(
                                po[:],
                                atb[h][ib][:, k4 * P:(k4 + 1) * P],
                                wo_sb[h][:],
                                start=(h == 0),
                                stop=(h == 1),
                            )
                        ost = osp.tile([P, NB], f32, tag="ost")
                        nc.vector.tensor_copy(ost[:], po[:])
                        nc.sync.dma_start(
                            out=o_ext[ic * P:(ic + 1) * P, :], in_=ost[:]
                        )

    nc.finalize()
    return nc


PREC = "f32r"


def _get_nc(seq_len=S, prec=None):
    prec = PREC if prec is None else prec
    key = ("nc", seq_len, prec)
    if key not in _CACHE:
        _CACHE[key] = _build(seq_len, prec)
    return _CACHE[key]


def _make_in_maps(x, Wq, bq, Wk, bk, Wv, bv, Wo):
    in_maps = []
    for c in range(8):
        b, hp = c // 4, c % 4
        m0 = hp * MW
        in_maps.append({
            "x_b": np.ascontiguousarray(x[b]),
            "wq": np.ascontiguousarray(Wq[:, m0:m0 + MW]),
            "wk": np.ascontiguousarray(Wk[:, m0:m0 + MW]),
            "wv": np.ascontiguousarray(Wv[:, m0:m0 + MW]),
            "wo": np.ascontiguousarray(Wo[m0:m0 + MW, :]),
            "bq": np.ascontiguousarray(bq[m0:m0 + MW].reshape(MW, 1)),
            "bk": np.ascontiguousarray(bk[m0:m0 + MW].reshape(MW, 1)),
            "bv": np.ascontiguousarray(bv[m0:m0 + MW].reshape(MW, 1)),
        })
    return in_maps


def _run(x, Wq, bq, Wk, bk, Wv, bv, Wo, bo, **spmd_kwargs):
    from concourse.bass_utils import run_bass_kernel_spmd

    args = [np.asarray(a, dtype=np.float32)
            for a in (x, Wq, bq, Wk, bk, Wv, bv, Wo, bo)]
    x, Wq, bq, Wk, bk, Wv, bv, Wo, bo = args
    nc = _get_nc()
    in_maps = _make_in_maps(x, Wq, bq, Wk, bk, Wv, bv, Wo)
    res = run_bass_kernel_spmd(nc, in_maps, list(range(8)), **spmd_kwargs)
    out = np.empty((B, S, D), dtype=np.float32)
    for b in range(B):
        acc = res.results[4 * b]["o_part"].astype(np.float32)
        for hp in range(1, 4):
            acc = acc + res.results[4 * b + hp]["o_part"]
        out[b] = acc + bo[None, :]
    return out, res


def kernel(x, Wq, bq, Wk, bk, Wv, bv, Wo, bo):
    out, _ = _run(x, Wq, bq, Wk, bk, Wv, bv, Wo, bo)
    return out
